# revision 35
# baseline (speedup 1.0000x reference)
"""Distributed Trainium2 kernel for nn_Attention (B=2, N=2048, C=1024, H=16, HD=64).

Sharding: (batch x head-group) parallel, ZERO device collectives.
Core c owns batch b=c//4 and heads [4*(c%4), 4*(c%4)+4).  Each core:
  - computes q,k (transposed layout) and v (natural layout) for its own
    4 heads over the FULL sequence (x^T for its batch is loaded whole),
  - applies RoPE to q,k on the vector engine (bf16, 4x DVE mode, with a
    partition-swapped signed-sin layout so both inputs share a base
    partition),
  - runs full 2048x2048 attention for its 4 heads (scores transposed,
    softmax denominators via an appended ones-column in v),
  - computes the PARTIAL output projection restricted to its 256 head
    dims, writing out^T [C, N] in bf16.
The host sums the 4 partial projections per batch while unsharding.

All matmuls bf16 with fp32 PSUM accumulation (tolerance 2e-2, measured
~8e-3).  Engine budget per core: PE ~139us of matmul rows, ACT ~133us
of softmax exp -- the emission order software-pipelines them: scores+exp
for a group (head, qc) run 3 groups ahead of that group's A@V, so the
scalar engine is saturated from ~15us on.
"""

import sys

if "/opt/trn_rl_repo" not in sys.path:
    sys.path.insert(0, "/opt/trn_rl_repo")

import numpy as np

B, N, C = 2, 2048, 1024
H, HD = 16, 64
NCORES = 8
GB = 4            # cores per batch
HPC = H // GB     # heads per core = 4
SC = HD ** -0.5   # attention scale
NC4 = N // 512    # 512-wide n windows
NC16 = N // 128   # 128-wide n windows (key chunks)


def build():
    import concourse.bass as bass
    import concourse.mybir as mybir
    import concourse.tile as tile
    from concourse import bacc

    f32 = mybir.dt.float32
    bf16 = mybir.dt.bfloat16
    AF = mybir.ActivationFunctionType

    nc = bacc.Bacc(None, target_bir_lowering=False, num_devices=NCORES)

    # ---- per-core external inputs (host pre-shards / pre-transposes) ----
    xT = nc.declare_dram_parameter("xT", [C, N], bf16, isOutput=False)
    wqk = nc.declare_dram_parameter("wqk", [C, 512], bf16, isOutput=False)
    wv = nc.declare_dram_parameter("wv", [C, 256], bf16, isOutput=False)
    wp = nc.declare_dram_parameter("wp", [256, C], bf16, isOutput=False)
    # cs[:, 0, :] = cos layout; cs[:, 1, :] = SWAPPED signed sin layout:
    # sin2[lo+i] = +sin[n, 32+i], sin2[lo+32+i] = -sin[n, i]
    cs = nc.declare_dram_parameter("cs", [128, 2 * N], bf16, isOutput=False)
    biasv = nc.declare_dram_parameter("biasv", [128, 8], f32, isOutput=False)
    out = nc.declare_dram_parameter("out", [C, N], bf16, isOutput=True)

    def mm(out_ap, lhsT_ap, rhs_ap, start, stop):
        nc.tensor.matmul(out_ap, lhsT_ap, rhs_ap, start=start, stop=stop)

    from contextlib import ExitStack

    with tile.TileContext(nc) as tc:
        with ExitStack() as stack:
            ep = stack.enter_context
            ep(nc.allow_low_precision(reason="bf16 attention, tol 2e-2"))
            constp = ep(tc.tile_pool(name="const", bufs=1))
            rawp = ep(tc.tile_pool(name="raw", bufs=3))
            tmpp = ep(tc.tile_pool(name="tmp", bufs=3))
            ptp = ep(tc.tile_pool(name="pt", bufs=48))
            rcpp = ep(tc.tile_pool(name="rcp", bufs=4))
            outp = ep(tc.tile_pool(name="outp", bufs=4))
            ps_a = ep(tc.tile_pool(name="ps_a", bufs=2, space="PSUM"))
            ps_s = ep(tc.tile_pool(name="ps_s", bufs=2, space="PSUM"))
            ps_av = ep(tc.tile_pool(name="ps_av", bufs=2, space="PSUM"))

            # ---- persistent SBUF ----
            # Few, large DMAs: each dma_start holds the shared HWDGE unit
            # ~630ns, so DMA count gates how early compute can start.
            # Order = first-use: k01 weights + first x window, cos/sin for
            # the first rope, remaining x windows, everything else.
            wqk_sb = constp.tile([128, 8, 512], bf16, name="wqk_sb")
            xT_sb = constp.tile([128, 8, N], bf16, name="xT_sb")
            cs_sb = constp.tile([128, 2, N], bf16, name="cs_sb")
            wqk_r = wqk.rearrange("(c p) d -> p c d", p=128)     # [128,8,512]
            xT_r = xT.rearrange("(c p) n -> p c n", p=128)       # [128,8,N]
            cs_r = cs.rearrange("p (a n) -> p a n", n=N)
            nc.sync.dma_start(wqk_sb[:, :, 256:384], wqk_r[:, :, 256:384])
            nc.sync.dma_start(xT_sb[:, 0:4, 0:512], xT_r[:, 0:4, 0:512])
            nc.sync.dma_start(xT_sb[:, 4:8, 0:512], xT_r[:, 4:8, 0:512])
            nc.sync.dma_start(cs_sb[:, 1, :], cs_r[:, 1, :])   # sin first
            nc.sync.dma_start(wqk_sb[:, :, 0:128], wqk_r[:, :, 0:128])
            nc.sync.dma_start(cs_sb[:, 0, :], cs_r[:, 0, :])
            for n4 in range(1, 4):
                nc.sync.dma_start(
                    xT_sb[:, :, n4 * 512:(n4 + 1) * 512],
                    xT_r[:, :, n4 * 512:(n4 + 1) * 512],
                )
            wv_sb = constp.tile([128, 8, 256], bf16, name="wv_sb")
            nc.sync.dma_start(
                wv_sb[:, :, :], wv.rearrange("(c p) d -> p c d", p=128)
            )
            nc.sync.dma_start(wqk_sb[:, :, 128:256], wqk_r[:, :, 128:256])
            nc.sync.dma_start(wqk_sb[:, :, 384:512], wqk_r[:, :, 384:512])
            wp_sb = constp.tile([128, 2, C], bf16, name="wp_sb")
            nc.sync.dma_start(
                wp_sb[:, :, :], wp.rearrange("(c p) d -> p c d", p=128)
            )
            bias_sb = constp.tile([128, 8], f32, name="bias_sb")
            nc.sync.dma_start(bias_sb[:, :], biasv[:, :])

            # roped q,k transposed: chunks 0=q01, 1=q23, 2=k01, 3=k23
            qk_sb = constp.tile([128, 4, N], bf16, name="qk_sb")
            # v natural: [n-part, n-chunk, head, dim(+ones)]
            v_sb = constp.tile([128, NC16, HPC, HD + 1], bf16, name="v_sb")
            nc.vector.memset(v_sb[:, :, :, HD:HD + 1], 1.0)
            # normalized attention out, natural: [q-part, qc, sub, head, dim]
            attn_sb = constp.tile([128, NC4, 4, HPC, HD], bf16, name="attn_sb")
            # attention out transposed: [dim-part, dim-chunk, n]
            attnT_sb = constp.tile([128, 2, N], bf16, name="attnT_sb")

            def qk_chunk(ch, n4):
                """qkv matmul + rope for one (128-dim q/k chunk, n window)."""
                nsl = slice(n4 * 512, (n4 + 1) * 512)
                ps = ps_a.tile([128, 512], f32, name="ps", tag="mm")
                for cc in range(8):
                    mm(ps[:, :], wqk_sb[:, cc, ch * 128:(ch + 1) * 128],
                       xT_sb[:, cc, nsl], cc == 0, cc == 7)
                raw = rawp.tile([128, 512], bf16, name="raw", tag="raw")
                nc.vector.tensor_copy(raw[:, :], ps[:, :])
                tmp = tmpp.tile([128, 512], bf16, name="tmp", tag="tmp")
                for lo in (0, 64):
                    nc.vector.tensor_mul(
                        tmp[lo:lo + 32, :],
                        raw[lo + 32:lo + 64, :],
                        cs_sb[lo + 32:lo + 64, 1, nsl],
                    )
                    nc.vector.tensor_mul(
                        tmp[lo + 32:lo + 64, :],
                        raw[lo:lo + 32, :],
                        cs_sb[lo:lo + 32, 1, nsl],
                    )
                nc.vector.tensor_mul(
                    qk_sb[:, ch, nsl], raw[:, :], cs_sb[:, 0, nsl]
                )
                nc.vector.tensor_add(
                    qk_sb[:, ch, nsl], qk_sb[:, ch, nsl], tmp[:, :]
                )

            def v_chunks(n16s):
                for n16 in n16s:
                    ps = ps_a.tile([128, 512], f32, name="ps", tag="mm")
                    for cc in range(8):
                        mm(ps[:, 0:256],
                           xT_sb[:, cc, n16 * 128:(n16 + 1) * 128],
                           wv_sb[:, cc, :], cc == 0, cc == 7)
                    nc.vector.tensor_copy(
                        v_sb[:, n16, :, 0:HD],
                        ps[:, 0:256].rearrange("p (h d) -> p h d", d=HD),
                    )

            pt_of = {}

            def scores_exp(h, qc, js=range(8)):
                qch, kch = h // 2, 2 + h // 2
                lo = (h % 2) * 64
                qsl = slice(qc * 512, (qc + 1) * 512)
                pts = pt_of.setdefault((h, qc), [])
                for j in js:
                    ps2 = ps_s.tile([128, 2, 512], f32, name="ps2", tag="sc")
                    for u in range(2):
                        kc = 2 * j + u
                        mm(ps2[:, u, :],
                           qk_sb[lo:lo + 64, kch, kc * 128:(kc + 1) * 128],
                           qk_sb[lo:lo + 64, qch, qsl], True, True)
                    pt = ptp.tile([128, 2, 512], bf16, name="pt", tag="pt")
                    nc.scalar.activation(
                        pt[:, :, :], ps2[:, :, :], AF.Exp, scale=SC
                    )
                    pts.append(pt)

            pending_proj = [None]

            def av_norm(h, qc, do_trans, seq=()):
                # seq: (h', qc', j) scores+exp emissions for upcoming
                # groups, interleaved at sub granularity so PE has fresh
                # matmuls while each po accumulator drains through the
                # reciprocal+normalize on DVE (a PE stall costs ~3us of
                # half-clock re-ramp, so density is everything)
                pts = pt_of.pop((h, qc))
                nj = len(seq)
                for sub in range(4):
                    for hh, qq, jj in seq[nj * sub // 4:nj * (sub + 1) // 4]:
                        scores_exp(hh, qq, js=(jj,))
                    if sub == 1 and pending_proj[0] is not None:
                        proj(pending_proj[0])
                        pending_proj[0] = None
                    po = ps_av.tile([128, HD + 1], f32, name="po", tag="av")
                    for kc in range(16):
                        mm(po[:, :],
                           pts[kc // 2][:, kc % 2,
                                        sub * 128:(sub + 1) * 128],
                           v_sb[:, kc, h, :], kc == 0, kc == 15)
                    rcp = rcpp.tile([128, 1], f32, name="rcp", tag="rcp")
                    nc.vector.reciprocal(rcp[:, :], po[:, HD:HD + 1])
                    nc.vector.tensor_scalar_mul(
                        attn_sb[:, qc, sub, h, :], po[:, 0:HD], rcp[:, :]
                    )
                    if do_trans:
                        # all heads of (qc, sub) now done: transpose eagerly
                        for dc in range(2):
                            nc.sync.dma_start_transpose(
                                attnT_sb[:, dc,
                                         qc * 512 + sub * 128:
                                         qc * 512 + (sub + 1) * 128],
                                attn_sb[:, qc, sub, 2 * dc:2 * dc + 2, :],
                            )

            def proj(qc):
                # sub-granular matmuls: sub windows 0-2 can run while the
                # last head's A@V for sub 3 is still in flight
                nsl = slice(qc * 512, (qc + 1) * 512)
                for cch in range(8):
                    ps = ps_a.tile([128, 512], f32, name="ps", tag="mm")
                    for sub in range(4):
                        ssl = slice(qc * 512 + sub * 128,
                                    qc * 512 + (sub + 1) * 128)
                        for dc in range(2):
                            mm(ps[:, sub * 128:(sub + 1) * 128],
                               wp_sb[:, dc, cch * 128:(cch + 1) * 128],
                               attnT_sb[:, dc, ssl], dc == 0, dc == 1)
                    osb = outp.tile([128, 512], bf16, name="osb", tag="osb")
                    nc.vector.tensor_scalar_add(
                        osb[:, :], ps[:, :], bias_sb[:, cch:cch + 1]
                    )
                    nc.gpsimd.dma_start(
                        out[cch * 128:(cch + 1) * 128, nsl], osb[:, :]
                    )

            # ---- emission schedule (software pipeline) ----
            # Heads 0,1 use q/k chunks (0,2); heads 2,3 use (1,3).  Group
            # order starts with heads 0,1 (their chunks are roped first)
            # and finishes each qc as early as possible so the four
            # projections spread across the run instead of piling up at
            # the end.  scores+exp stay ~4 groups ahead of their A@V so
            # the scalar engine (exp, co-critical with PE) never starves.
            groups = [(0, 0), (1, 0), (0, 1), (1, 1),
                      (2, 0), (3, 0), (2, 1), (3, 1),
                      (0, 2), (1, 2), (2, 2), (3, 2),
                      (0, 3), (1, 3), (2, 3), (3, 3)]
            heads_left = {qc: HPC for qc in range(NC4)}
            emitted = [0]

            def emit_se_until(tgt):
                while emitted[0] < min(tgt, len(groups)):
                    scores_exp(*groups[emitted[0]])
                    emitted[0] += 1

            # Phase 1: heads 0,1's q/k windows; scores j-tiles for the
            # first two groups follow each key window they depend on, so
            # the exp stream starts ~12us in instead of after all of qkv.
            # The v matmuls (no exp consumer behind them) go LAST, after
            # six scores groups are queued, so the scalar engine never
            # starves while PE grinds through them.
            # scores j-pairs lag their key window by one, so PE has the
            # next window's qkv matmuls to chew on during each rope
            for n4 in range(NC4):
                qk_chunk(2, n4)
                qk_chunk(0, n4)
                if n4 > 0:
                    scores_exp(0, 0, js=(2 * n4 - 2, 2 * n4 - 1))
                    scores_exp(1, 0, js=(2 * n4 - 2, 2 * n4 - 1))
            scores_exp(0, 0, js=(6, 7))
            scores_exp(1, 0, js=(6, 7))
            scores_exp(0, 1)
            scores_exp(1, 1)
            for n4 in range(NC4):
                qk_chunk(1, n4)
            for n4 in range(NC4):
                qk_chunk(3, n4)
            scores_exp(2, 0)
            scores_exp(3, 0)
            v_chunks(range(4))
            scores_exp(2, 1, js=range(4))
            v_chunks(range(4, 8))
            scores_exp(2, 1, js=range(4, 8))
            v_chunks(range(8, 16))
            emitted[0] = 7
            for i, (h, qc) in enumerate(groups):
                tgt = min(len(groups), i + 5 + max(0, i - 9))
                seq = [(groups[gi][0], groups[gi][1], j)
                       for gi in range(emitted[0], tgt) for j in range(8)]
                emitted[0] = max(emitted[0], tgt)
                heads_left[qc] -= 1
                av_norm(h, qc, do_trans=heads_left[qc] == 0, seq=seq)
                if heads_left[qc] == 0:
                    # defer the projection one iteration: its transposes
                    # then finish well before PE reaches the proj matmuls
                    if pending_proj[0] is not None:
                        proj(pending_proj[0])
                    pending_proj[0] = qc
            proj(pending_proj[0])

    nc.compile()
    return nc


_NC_CACHE = {}


def _get_nc():
    if "nc" not in _NC_CACHE:
        _NC_CACHE["nc"] = build()
    return _NC_CACHE["nc"]


def make_in_maps(x, cos, sin, qkv_w, proj_w, proj_b):
    import ml_dtypes

    bf16 = ml_dtypes.bfloat16
    x = np.asarray(x, np.float32)
    cos = np.asarray(cos, np.float32)
    sin = np.asarray(sin, np.float32)
    qkv_w = np.asarray(qkv_w, np.float32)
    proj_w = np.asarray(proj_w, np.float32)
    proj_b = np.asarray(proj_b, np.float32)

    sign = np.concatenate([-np.ones(32, np.float32), np.ones(32, np.float32)])
    cosT = cos.T                      # [HD, N]
    sinT = (sin * sign).T             # [HD, N] signed
    # swapped signed sin: row lo+i -> +sin[:, 32+i], row lo+32+i -> -sin[:, i]
    sin_swap = np.concatenate([sinT[32:64], sinT[0:32]], 0)
    cos2v = np.concatenate([cosT, cosT], 0)                # [128, N]
    sin2v = np.concatenate([sin_swap, sin_swap], 0)
    csv = np.ascontiguousarray(
        np.concatenate([cos2v, sin2v], 1).astype(bf16))    # [128, 2N]

    in_maps = []
    for c in range(NCORES):
        b, g = c // GB, c % GB
        h0 = HPC * g                  # first head of this core
        d0 = h0 * HD                  # first q/k/v row block
        wqk_cols = np.concatenate(
            [
                qkv_w[d0:d0 + 128],                    # q heads h0, h0+1
                qkv_w[d0 + 128:d0 + 256],              # q heads h0+2, h0+3
                qkv_w[C + d0:C + d0 + 128],            # k heads h0, h0+1
                qkv_w[C + d0 + 128:C + d0 + 256],      # k heads h0+2, h0+3
            ],
            axis=0,
        )
        wqkT = np.ascontiguousarray(wqk_cols.T.astype(bf16))   # [C, 512]
        wvT = np.ascontiguousarray(
            qkv_w[2 * C + d0:2 * C + d0 + 256].T.astype(bf16))  # [C, 256]
        wpT = np.ascontiguousarray(
            proj_w[:, d0:d0 + 256].T.astype(bf16))              # [256, C]
        xTc = np.ascontiguousarray(x[b].T.astype(bf16))         # [C, N]
        bv = np.zeros((128, 8), np.float32)
        if g == 0:
            bv[:] = proj_b.reshape(8, 128).T
        in_maps.append(
            {
                "xT": xTc,
                "wqk": wqkT,
                "wv": wvT,
                "wp": wpT,
                "cs": csv,
                "biasv": bv,
            }
        )
    return in_maps


def assemble(results):
    out = np.zeros((B, N, C), np.float32)
    for c in range(NCORES):
        b = c // GB
        out[b] += results[c]["out"].T.astype(np.float32)
    return out


def kernel(x, cos, sin, qkv_w, proj_w, proj_b):
    from concourse.bass_utils import run_bass_kernel_spmd

    nc = _get_nc()
    in_maps = make_in_maps(x, cos, sin, qkv_w, proj_w, proj_b)
    res = run_bass_kernel_spmd(nc, in_maps, core_ids=list(range(NCORES)))
    return assemble(res.results)


# revision 39
# speedup vs baseline: 1.0283x; 1.0283x over previous
"""Distributed Trainium2 kernel for nn_Attention (B=2, N=2048, C=1024, H=16, HD=64).

Sharding: (batch x head-group) parallel, ZERO device collectives.
Core c owns batch b=c//4 and heads [4*(c%4), 4*(c%4)+4).  Each core:
  - computes q,k (transposed layout) and v (natural layout) for its own
    4 heads over the FULL sequence (x^T for its batch is loaded whole),
  - applies RoPE to q,k on the vector engine (bf16, 4x DVE mode, with a
    partition-swapped signed-sin layout so both inputs share a base
    partition),
  - runs full 2048x2048 attention for its 4 heads (scores transposed,
    softmax denominators via an appended ones-column in v),
  - computes the PARTIAL output projection restricted to its 256 head
    dims, writing out^T [C, N] in bf16.
The host sums the 4 partial projections per batch while unsharding.

All matmuls bf16 with fp32 PSUM accumulation (tolerance 2e-2, measured
~8e-3).  Engine budget per core: PE ~139us of matmul rows, ACT ~133us
of softmax exp -- the emission order software-pipelines them: scores+exp
for a group (head, qc) run 3 groups ahead of that group's A@V, so the
scalar engine is saturated from ~15us on.
"""

import sys

if "/opt/trn_rl_repo" not in sys.path:
    sys.path.insert(0, "/opt/trn_rl_repo")

import numpy as np

B, N, C = 2, 2048, 1024
H, HD = 16, 64
NCORES = 8
GB = 4            # cores per batch
HPC = H // GB     # heads per core = 4
SC = HD ** -0.5   # attention scale
NC4 = N // 512    # 512-wide n windows
NC16 = N // 128   # 128-wide n windows (key chunks)


def build():
    import concourse.bass as bass
    import concourse.mybir as mybir
    import concourse.tile as tile
    from concourse import bacc

    f32 = mybir.dt.float32
    bf16 = mybir.dt.bfloat16
    AF = mybir.ActivationFunctionType

    nc = bacc.Bacc(None, target_bir_lowering=False, num_devices=NCORES)

    # ---- per-core external inputs (host pre-shards / pre-transposes) ----
    xT = nc.declare_dram_parameter("xT", [C, N], bf16, isOutput=False)
    wqk = nc.declare_dram_parameter("wqk", [C, 512], bf16, isOutput=False)
    wv = nc.declare_dram_parameter("wv", [C, 256], bf16, isOutput=False)
    wp = nc.declare_dram_parameter("wp", [256, C], bf16, isOutput=False)
    # cs[:, 0, :] = cos layout; cs[:, 1, :] = SWAPPED signed sin layout:
    # sin2[lo+i] = +sin[n, 32+i], sin2[lo+32+i] = -sin[n, i]
    cs = nc.declare_dram_parameter("cs", [128, 2 * N], bf16, isOutput=False)
    biasv = nc.declare_dram_parameter("biasv", [128, 8], f32, isOutput=False)
    out = nc.declare_dram_parameter("out", [C, N], bf16, isOutput=True)

    def mm(out_ap, lhsT_ap, rhs_ap, start, stop):
        nc.tensor.matmul(out_ap, lhsT_ap, rhs_ap, start=start, stop=stop)

    from contextlib import ExitStack

    with tile.TileContext(nc) as tc:
        with ExitStack() as stack:
            ep = stack.enter_context
            ep(nc.allow_low_precision(reason="bf16 attention, tol 2e-2"))
            constp = ep(tc.tile_pool(name="const", bufs=1))
            rawp = ep(tc.tile_pool(name="raw", bufs=3))
            tmpp = ep(tc.tile_pool(name="tmp", bufs=3))
            # x^T, cos/sin and the exp outputs share one pool+tag: the 20
            # x/cs tiles release their slots once qkv+rope consume them,
            # deepening the exp-ahead pipeline to 8 groups mid-run
            ptp = ep(tc.tile_pool(name="pt", bufs=68))
            rcpp = ep(tc.tile_pool(name="rcp", bufs=4))
            outp = ep(tc.tile_pool(name="outp", bufs=4))
            ps_a = ep(tc.tile_pool(name="ps_a", bufs=2, space="PSUM"))
            ps_s = ep(tc.tile_pool(name="ps_s", bufs=2, space="PSUM"))
            ps_av = ep(tc.tile_pool(name="ps_av", bufs=2, space="PSUM"))

            # ---- persistent SBUF ----
            # Few, large DMAs: each dma_start holds the shared HWDGE unit
            # ~630ns, so DMA count gates how early compute can start.
            # Order = first-use: k01 weights + first x window, cos/sin for
            # the first rope, remaining x windows, everything else.
            wqk_sb = constp.tile([128, 8, 512], bf16, name="wqk_sb")
            wqk_r = wqk.rearrange("(c p) d -> p c d", p=128)     # [128,8,512]
            xT_r = xT.rearrange("(c p) n -> p c n", p=128)       # [128,8,N]
            cs_r = cs.rearrange("p (a n) -> p a n", n=N)
            # x^T as 16 pool tiles [(cc-pair, window)], cs as 4 [(window)]
            xt_t, cs_t = {}, {}

            def load_xt_window(n4):
                nsl = slice(n4 * 512, (n4 + 1) * 512)
                for p in range(4):
                    t = ptp.tile([128, 2, 512], bf16, name="pt", tag="pt")
                    nc.sync.dma_start(t[:, :, :], xT_r[:, 2 * p:2 * p + 2,
                                                       nsl])
                    xt_t[(p, n4)] = t

            def xs(cc, n4):     # [128, 512] window slice for chunk cc
                return xt_t[(cc // 2, n4)][:, cc % 2, :]

            nc.sync.dma_start(wqk_sb[:, :, 256:384], wqk_r[:, :, 256:384])
            load_xt_window(0)
            cs_t[0] = ptp.tile([128, 2, 512], bf16, name="pt", tag="pt")
            nc.sync.dma_start(cs_t[0][:, :, :], cs_r[:, :, 0:512])
            nc.sync.dma_start(wqk_sb[:, :, 0:128], wqk_r[:, :, 0:128])
            for n4 in range(1, 4):
                load_xt_window(n4)
                cs_t[n4] = ptp.tile([128, 2, 512], bf16, name="pt", tag="pt")
                nc.sync.dma_start(
                    cs_t[n4][:, :, :],
                    cs_r[:, :, n4 * 512:(n4 + 1) * 512],
                )
            wv_sb = constp.tile([128, 8, 256], bf16, name="wv_sb")
            nc.sync.dma_start(
                wv_sb[:, :, :], wv.rearrange("(c p) d -> p c d", p=128)
            )
            nc.sync.dma_start(wqk_sb[:, :, 128:256], wqk_r[:, :, 128:256])
            nc.sync.dma_start(wqk_sb[:, :, 384:512], wqk_r[:, :, 384:512])
            wp_sb = constp.tile([128, 2, C], bf16, name="wp_sb")
            nc.sync.dma_start(
                wp_sb[:, :, :], wp.rearrange("(c p) d -> p c d", p=128)
            )
            bias_sb = constp.tile([128, 8], f32, name="bias_sb")
            nc.sync.dma_start(bias_sb[:, :], biasv[:, :])

            # roped q,k transposed: chunks 0=q01, 1=q23, 2=k01, 3=k23
            qk_sb = constp.tile([128, 4, N], bf16, name="qk_sb")
            # v natural: [n-part, n-chunk, head, dim(+ones)]
            v_sb = constp.tile([128, NC16, HPC, HD + 1], bf16, name="v_sb")
            nc.vector.memset(v_sb[:, :, :, HD:HD + 1], 1.0)
            # normalized attention out, natural: [q-part, qc, sub, head, dim]
            attn_sb = constp.tile([128, NC4, 4, HPC, HD], bf16, name="attn_sb")
            # attention out transposed: [dim-part, dim-chunk, n]
            attnT_sb = constp.tile([128, 2, N], bf16, name="attnT_sb")

            def qk_chunk(ch, n4):
                """qkv matmul + rope for one (128-dim q/k chunk, n window)."""
                nsl = slice(n4 * 512, (n4 + 1) * 512)
                ct = cs_t[n4]
                ps = ps_a.tile([128, 512], f32, name="ps", tag="mm")
                for cc in range(8):
                    mm(ps[:, :], wqk_sb[:, cc, ch * 128:(ch + 1) * 128],
                       xs(cc, n4), cc == 0, cc == 7)
                raw = rawp.tile([128, 512], bf16, name="raw", tag="raw")
                nc.vector.tensor_copy(raw[:, :], ps[:, :])
                tmp = tmpp.tile([128, 512], bf16, name="tmp", tag="tmp")
                for lo in (0, 64):
                    nc.vector.tensor_mul(
                        tmp[lo:lo + 32, :],
                        raw[lo + 32:lo + 64, :],
                        ct[lo + 32:lo + 64, 1, :],
                    )
                    nc.vector.tensor_mul(
                        tmp[lo + 32:lo + 64, :],
                        raw[lo:lo + 32, :],
                        ct[lo:lo + 32, 1, :],
                    )
                nc.vector.tensor_mul(
                    qk_sb[:, ch, nsl], raw[:, :], ct[:, 0, :]
                )
                nc.vector.tensor_add(
                    qk_sb[:, ch, nsl], qk_sb[:, ch, nsl], tmp[:, :]
                )

            def v_chunks(n16s):
                for n16 in n16s:
                    ps = ps_a.tile([128, 512], f32, name="ps", tag="mm")
                    for cc in range(8):
                        mm(ps[:, 0:256],
                           xt_t[(cc // 2, n16 // 4)][
                               :, cc % 2,
                               (n16 % 4) * 128:(n16 % 4 + 1) * 128],
                           wv_sb[:, cc, :], cc == 0, cc == 7)
                    nc.vector.tensor_copy(
                        v_sb[:, n16, :, 0:HD],
                        ps[:, 0:256].rearrange("p (h d) -> p h d", d=HD),
                    )

            pt_of = {}

            def scores_exp(h, qc, js=range(8)):
                qch, kch = h // 2, 2 + h // 2
                lo = (h % 2) * 64
                qsl = slice(qc * 512, (qc + 1) * 512)
                pts = pt_of.setdefault((h, qc), [])
                for j in js:
                    ps2 = ps_s.tile([128, 2, 512], f32, name="ps2", tag="sc")
                    for u in range(2):
                        kc = 2 * j + u
                        mm(ps2[:, u, :],
                           qk_sb[lo:lo + 64, kch, kc * 128:(kc + 1) * 128],
                           qk_sb[lo:lo + 64, qch, qsl], True, True)
                    pt = ptp.tile([128, 2, 512], bf16, name="pt", tag="pt")
                    nc.scalar.activation(
                        pt[:, :, :], ps2[:, :, :], AF.Exp, scale=SC
                    )
                    pts.append(pt)

            pending_proj = [None]

            def av_norm(h, qc, do_trans, seq=()):
                # seq: (h', qc', j) scores+exp emissions for upcoming
                # groups, interleaved at sub granularity so PE has fresh
                # matmuls while each po accumulator drains through the
                # reciprocal+normalize on DVE (a PE stall costs ~3us of
                # half-clock re-ramp, so density is everything)
                pts = pt_of.pop((h, qc))
                nj = len(seq)
                for sub in range(4):
                    for hh, qq, jj in seq[nj * sub // 4:nj * (sub + 1) // 4]:
                        scores_exp(hh, qq, js=(jj,))
                    if sub == 1 and pending_proj[0] is not None:
                        proj(pending_proj[0])
                        pending_proj[0] = None
                    po = ps_av.tile([128, HD + 1], f32, name="po", tag="av")
                    for kc in range(16):
                        mm(po[:, :],
                           pts[kc // 2][:, kc % 2,
                                        sub * 128:(sub + 1) * 128],
                           v_sb[:, kc, h, :], kc == 0, kc == 15)
                    rcp = rcpp.tile([128, 1], f32, name="rcp", tag="rcp")
                    nc.vector.reciprocal(rcp[:, :], po[:, HD:HD + 1])
                    nc.vector.tensor_scalar_mul(
                        attn_sb[:, qc, sub, h, :], po[:, 0:HD], rcp[:, :]
                    )
                    if do_trans:
                        # all heads of (qc, sub) now done: transpose eagerly
                        for dc in range(2):
                            nc.sync.dma_start_transpose(
                                attnT_sb[:, dc,
                                         qc * 512 + sub * 128:
                                         qc * 512 + (sub + 1) * 128],
                                attn_sb[:, qc, sub, 2 * dc:2 * dc + 2, :],
                            )

            def proj(qc):
                # sub-granular matmuls: sub windows 0-2 can run while the
                # last head's A@V for sub 3 is still in flight
                nsl = slice(qc * 512, (qc + 1) * 512)
                for cch in range(8):
                    ps = ps_a.tile([128, 512], f32, name="ps", tag="mm")
                    for sub in range(4):
                        ssl = slice(qc * 512 + sub * 128,
                                    qc * 512 + (sub + 1) * 128)
                        for dc in range(2):
                            mm(ps[:, sub * 128:(sub + 1) * 128],
                               wp_sb[:, dc, cch * 128:(cch + 1) * 128],
                               attnT_sb[:, dc, ssl], dc == 0, dc == 1)
                    osb = outp.tile([128, 512], bf16, name="osb", tag="osb")
                    nc.vector.tensor_scalar_add(
                        osb[:, :], ps[:, :], bias_sb[:, cch:cch + 1]
                    )
                    nc.gpsimd.dma_start(
                        out[cch * 128:(cch + 1) * 128, nsl], osb[:, :]
                    )

            # ---- emission schedule (software pipeline) ----
            # Heads 0,1 use q/k chunks (0,2); heads 2,3 use (1,3).  Group
            # order starts with heads 0,1 (their chunks are roped first)
            # and finishes each qc as early as possible so the four
            # projections spread across the run instead of piling up at
            # the end.  scores+exp stay ~4 groups ahead of their A@V so
            # the scalar engine (exp, co-critical with PE) never starves.
            groups = [(0, 0), (1, 0), (0, 1), (1, 1),
                      (2, 0), (3, 0), (2, 1), (3, 1),
                      (0, 2), (1, 2), (2, 2), (3, 2),
                      (0, 3), (1, 3), (2, 3), (3, 3)]
            heads_left = {qc: HPC for qc in range(NC4)}
            emitted = [0]

            def emit_se_until(tgt):
                while emitted[0] < min(tgt, len(groups)):
                    scores_exp(*groups[emitted[0]])
                    emitted[0] += 1

            # Phase 1: heads 0,1's q/k windows; scores j-tiles for the
            # first two groups follow each key window they depend on, so
            # the exp stream starts ~12us in instead of after all of qkv.
            # The v matmuls (no exp consumer behind them) go LAST, after
            # six scores groups are queued, so the scalar engine never
            # starves while PE grinds through them.
            # scores j-pairs lag their key window by one, so PE has the
            # next window's qkv matmuls to chew on during each rope
            for n4 in range(NC4):
                qk_chunk(2, n4)
                qk_chunk(0, n4)
                if n4 > 0:
                    scores_exp(0, 0, js=(2 * n4 - 2, 2 * n4 - 1))
                    scores_exp(1, 0, js=(2 * n4 - 2, 2 * n4 - 1))
            scores_exp(0, 0, js=(6, 7))
            scores_exp(1, 0, js=(6, 7))
            scores_exp(0, 1)
            scores_exp(1, 1)
            for n4 in range(NC4):
                qk_chunk(1, n4)
            for n4 in range(NC4):
                qk_chunk(3, n4)
            scores_exp(2, 0)
            scores_exp(3, 0)
            v_chunks(range(4))
            scores_exp(2, 1, js=range(4))
            v_chunks(range(4, 8))
            scores_exp(2, 1, js=range(4, 8))
            v_chunks(range(8, 16))
            emitted[0] = 7
            for i, (h, qc) in enumerate(groups):
                tgt = min(len(groups), i + 5 + max(0, i - 9))
                seq = [(groups[gi][0], groups[gi][1], j)
                       for gi in range(emitted[0], tgt) for j in range(8)]
                emitted[0] = max(emitted[0], tgt)
                heads_left[qc] -= 1
                av_norm(h, qc, do_trans=heads_left[qc] == 0, seq=seq)
                if heads_left[qc] == 0:
                    # defer the projection one iteration: its transposes
                    # then finish well before PE reaches the proj matmuls
                    if pending_proj[0] is not None:
                        proj(pending_proj[0])
                    pending_proj[0] = qc
            proj(pending_proj[0])

    nc.compile()
    return nc


_NC_CACHE = {}


def _get_nc():
    if "nc" not in _NC_CACHE:
        _NC_CACHE["nc"] = build()
    return _NC_CACHE["nc"]


def make_in_maps(x, cos, sin, qkv_w, proj_w, proj_b):
    import ml_dtypes

    bf16 = ml_dtypes.bfloat16
    x = np.asarray(x, np.float32)
    cos = np.asarray(cos, np.float32)
    sin = np.asarray(sin, np.float32)
    qkv_w = np.asarray(qkv_w, np.float32)
    proj_w = np.asarray(proj_w, np.float32)
    proj_b = np.asarray(proj_b, np.float32)

    sign = np.concatenate([-np.ones(32, np.float32), np.ones(32, np.float32)])
    cosT = cos.T                      # [HD, N]
    sinT = (sin * sign).T             # [HD, N] signed
    # swapped signed sin: row lo+i -> +sin[:, 32+i], row lo+32+i -> -sin[:, i]
    sin_swap = np.concatenate([sinT[32:64], sinT[0:32]], 0)
    cos2v = np.concatenate([cosT, cosT], 0)                # [128, N]
    sin2v = np.concatenate([sin_swap, sin_swap], 0)
    csv = np.ascontiguousarray(
        np.concatenate([cos2v, sin2v], 1).astype(bf16))    # [128, 2N]

    in_maps = []
    for c in range(NCORES):
        b, g = c // GB, c % GB
        h0 = HPC * g                  # first head of this core
        d0 = h0 * HD                  # first q/k/v row block
        wqk_cols = np.concatenate(
            [
                qkv_w[d0:d0 + 128],                    # q heads h0, h0+1
                qkv_w[d0 + 128:d0 + 256],              # q heads h0+2, h0+3
                qkv_w[C + d0:C + d0 + 128],            # k heads h0, h0+1
                qkv_w[C + d0 + 128:C + d0 + 256],      # k heads h0+2, h0+3
            ],
            axis=0,
        )
        wqkT = np.ascontiguousarray(wqk_cols.T.astype(bf16))   # [C, 512]
        wvT = np.ascontiguousarray(
            qkv_w[2 * C + d0:2 * C + d0 + 256].T.astype(bf16))  # [C, 256]
        wpT = np.ascontiguousarray(
            proj_w[:, d0:d0 + 256].T.astype(bf16))              # [256, C]
        xTc = np.ascontiguousarray(x[b].T.astype(bf16))         # [C, N]
        bv = np.zeros((128, 8), np.float32)
        if g == 0:
            bv[:] = proj_b.reshape(8, 128).T
        in_maps.append(
            {
                "xT": xTc,
                "wqk": wqkT,
                "wv": wvT,
                "wp": wpT,
                "cs": csv,
                "biasv": bv,
            }
        )
    return in_maps


def assemble(results):
    out = np.zeros((B, N, C), np.float32)
    for c in range(NCORES):
        b = c // GB
        out[b] += results[c]["out"].T.astype(np.float32)
    return out


def kernel(x, cos, sin, qkv_w, proj_w, proj_b):
    from concourse.bass_utils import run_bass_kernel_spmd

    nc = _get_nc()
    in_maps = make_in_maps(x, cos, sin, qkv_w, proj_w, proj_b)
    res = run_bass_kernel_spmd(nc, in_maps, core_ids=list(range(NCORES)))
    return assemble(res.results)
